# revision 7
# baseline (speedup 1.0000x reference)
"""Trainium2 Bass kernel for nn_Attention (dense transformer attention block).

Full causal attention: QKV projection + RoPE + softmax(QK^T/sqrt(d) + mask)V + WO,
bsz=1, seqlen=2048, dim=4096, 32 heads x head_dim 128, fp32 I/O.

Sharding: tensor-parallel across heads on 8 NeuronCores. Core c owns heads
4c..4c+3 (wq/wk/wv output columns, attention) and wo output columns
512c..512c+512 (after an AllGather of the per-core attn^T shard along the
head axis). Host concatenates the 8 output column shards.

v2 layout: Q, K AND V projections all run against a fully SBUF-resident x^T
(single 16.8 MB load, consumed at PE rate from the first tile via a
Q0/K0-interleaved start). All matmuls are bf16 (the RoPE half-swap and the
softmax denominator run as bf16 PE matmuls). The final AllGather is split in
two head-halves so the last WO tile overlaps the gather.
"""

import ml_dtypes
import numpy as np

import concourse.bacc as bacc
import concourse.mybir as mybir
import concourse.tile as tile
from concourse.bass_utils import run_bass_kernel_spmd

# Problem constants (hardcoded per contract)
N_CORES = 8
S = 2048              # sequence length
D = 4096              # model dim
HD = 128              # head dim
NH_LOC = 4            # heads per core
DSH = 512             # per-core shard width (NH_LOC * HD)
KT = D // 128         # 32 contraction tiles over model dim
QTILES = S // 128     # 16 token tiles
QRANGES = S // 512    # 4 query ranges of 512
SCALE = float(1.0 / np.sqrt(HD))

F32 = mybir.dt.float32
BF16 = mybir.dt.bfloat16

_PROGRAMS = {}


def _build_program(mode):
    """mode: 'causal' (triu -1e9 mask), 'nomask' (zero mask), 'general'
    (arbitrary additive mask streamed from DRAM)."""
    causal = mode == "causal"
    general = mode == "general"

    nc = bacc.Bacc("TRN2", target_bir_lowering=False, debug=False,
                   num_devices=N_CORES)

    # ---- external inputs (per core) ----
    xT_d = nc.dram_tensor("xT", [D, S], BF16, kind="ExternalInput")
    wq_d = nc.dram_tensor("wq", [NH_LOC, 128, KT, HD], BF16, kind="ExternalInput")
    wk_d = nc.dram_tensor("wk", [NH_LOC, 128, KT, HD], BF16, kind="ExternalInput")
    wv_d = nc.dram_tensor("wv", [128, KT, DSH], BF16, kind="ExternalInput")
    wo_d = nc.dram_tensor("wo", [128, KT, DSH], BF16, kind="ExternalInput")
    fr_d = nc.dram_tensor("fr128", [128, S], BF16, kind="ExternalInput")
    fis_d = nc.dram_tensor("fis128", [128, S], BF16, kind="ExternalInput")
    perm_d = nc.dram_tensor("perm", [128, 128], BF16, kind="ExternalInput")
    onesmat_d = nc.dram_tensor("onesmat", [128, 128], BF16, kind="ExternalInput")
    if causal:
        maskt_d = nc.dram_tensor("maskt", [128, 128], F32, kind="ExternalInput")
    if general:
        masktf_d = nc.dram_tensor("masktf", [S, S], F32, kind="ExternalInput")
    out_d = nc.dram_tensor("out", [S, DSH], F32, kind="ExternalOutput")

    with tile.TileContext(nc) as tc:
        with (
            tc.tile_pool(name="consts", bufs=1) as cns,
            tc.tile_pool(name="dram", bufs=1, space="DRAM") as dram,
            tc.tile_pool(name="akv", bufs=1) as akv,
            tc.tile_pool(name="ps", bufs=1, space="PSUM") as ps,
        ):
            qt_spill = dram.tile([DSH, S], BF16)    # Q^T rotated, [d, s]
            agi = [dram.tile([DSH, 512], BF16, name=f"agi{r}") for r in range(4)]
            # full gathers for qr 0..2, split halves for qr 3
            ago = [dram.tile([D, 512], BF16, addr_space="Shared", name=f"ago{r}")
                   for r in range(3)]
            ago3 = [dram.tile([D // 2, 512], BF16, addr_space="Shared",
                              name=f"ago3{hf}") for hf in range(2)]

            onesmat_sb = cns.tile([128, 128], BF16, tag="om")
            if causal:
                maskt_sb = cns.tile([128, 128], F32, tag="maskt")

            kts = [akv.tile([128, S], BF16, tag=f"kth{h}", name=f"kth{h}")
                   for h in range(NH_LOC)]
            # all heads' V: [k-token part, token tile, 4 heads * 128 hd]
            vhs = akv.tile([128, QTILES, DSH], BF16, tag="vhs", name="vhs")

            # ---------- Section 1: Q/K/V projections (x^T fully resident) ----
            anchor = [None]
            with (
                tc.tile_pool(name="p1c", bufs=1) as p1c,
                tc.tile_pool(name="xtp", bufs=1) as xtp,
                tc.tile_pool(name="qkw", bufs=2) as qkw,
                tc.tile_pool(name="qkd", bufs=2) as qkd,
                tc.tile_pool(name="vw", bufs=2) as vw,
            ):
                perm_sb = p1c.tile([128, 128], BF16, tag="perm")
                fr_sb = p1c.tile([128, S], BF16, tag="fr")
                fis_sb = p1c.tile([128, S], BF16, tag="fis")

                xt_sb = xtp.tile([128, KT, S], BF16, tag="xt")
                # fine-grained early chunks so the first matmuls start ~3us in
                xt_chunks = [(0, 1), (1, 2), (2, 3), (3, 4), (4, 6), (6, 8),
                             (8, 11), (11, 15), (15, 20), (20, 26), (26, 32)]
                for ch, (k0, k1) in enumerate(xt_chunks):
                    nc.sync.dma_start(
                        xt_sb[:, k0:k1, :],
                        xT_d[k0 * 128:k1 * 128, :]
                        .rearrange("(kt p) s -> p kt s", p=128),
                    )
                    if ch == 0:
                        nc.scalar.dma_start(perm_sb[:], perm_d[:, :])
                        nc.scalar.dma_start(fr_sb[:], fr_d[:, :])
                        nc.scalar.dma_start(fis_sb[:], fis_d[:, :])
                        nc.scalar.dma_start(onesmat_sb[:], onesmat_d[:, :])
                        if causal:
                            nc.scalar.dma_start(maskt_sb[:], maskt_d[:, :])

                def emit_rope(head, psums, is_q, tagset):
                    """Evacuate 4 psum token-blocks of a projected head,
                    apply RoPE, write Q to the DRAM spill / K to kts."""
                    for j in range(4):
                        qt_bf = qkd.tile([128, 512], BF16, tag="qt")
                        nc.scalar.copy(qt_bf[:], psums[j][:])
                        swap_ps = ps.tile([128, 512], F32, tag=f"{tagset}{j}",
                                          name=f"swap{head}_{is_q}_{j}",
                                          bufs=1)
                        sw = nc.tensor.matmul(swap_ps[:], perm_sb[:], qt_bf[:])
                        if head == 3 and not is_q and j == 3:
                            anchor[0] = sw
                        t1 = qkd.tile([128, 512], F32, tag="t1")
                        nc.vector.tensor_mul(
                            t1[:], qt_bf[:], fr_sb[:, j * 512:(j + 1) * 512])
                        t2 = qkd.tile([128, 512], F32, tag="t2")
                        nc.vector.tensor_mul(
                            t2[:], swap_ps[:],
                            fis_sb[:, j * 512:(j + 1) * 512])
                        if is_q:
                            rot = qkd.tile([128, 512], BF16, tag="rot")
                            nc.vector.tensor_add(rot[:], t1[:], t2[:])
                            nc.sync.dma_start(
                                qt_spill[head * 128:(head + 1) * 128,
                                         j * 512:(j + 1) * 512],
                                rot[:],
                            )
                        else:
                            nc.vector.tensor_add(
                                kts[head][:, j * 512:(j + 1) * 512],
                                t1[:], t2[:])

                # Q/K per head, interleaved per-kt so the PE rides the x^T DMA
                for head in range(NH_LOC):
                    psQ = [ps.tile([128, 512], F32, tag=f"a{j}",
                                   name=f"qps{head}_{j}", bufs=1)
                           for j in range(4)]
                    psK = [ps.tile([128, 512], F32, tag=f"b{j}",
                                   name=f"kps{head}_{j}", bufs=1)
                           for j in range(4)]
                    for wc in range(4):
                        wq_c = qkw.tile([128, 8, 128], BF16, tag="wq")
                        nc.scalar.dma_start(
                            wq_c[:], wq_d[head, :, wc * 8:(wc + 1) * 8, :])
                        wk_c = qkw.tile([128, 8, 128], BF16, tag="wk")
                        nc.scalar.dma_start(
                            wk_c[:], wk_d[head, :, wc * 8:(wc + 1) * 8, :])
                        for kt8 in range(8):
                            kt = wc * 8 + kt8
                            for j in range(4):
                                nc.tensor.matmul(
                                    psQ[j][:], wq_c[:, kt8, :],
                                    xt_sb[:, kt, j * 512:(j + 1) * 512],
                                    start=(kt == 0), stop=(kt == KT - 1))
                            for j in range(4):
                                nc.tensor.matmul(
                                    psK[j][:], wk_c[:, kt8, :],
                                    xt_sb[:, kt, j * 512:(j + 1) * 512],
                                    start=(kt == 0), stop=(kt == KT - 1))
                    emit_rope(head, psQ, True, "a")
                    emit_rope(head, psK, False, "b")

                # V projection from resident x^T
                for vq in range(4):
                    psv = [ps.tile([128, 512], F32, tag=f"a{t}",
                                   name=f"vps{vq}_{t}", bufs=1)
                           for t in range(4)]
                    for ktc in range(8):
                        wv_c = vw.tile([128, 4, 512], BF16, tag="wv")
                        nc.scalar.dma_start(
                            wv_c[:], wv_d[:, ktc * 4:(ktc + 1) * 4, :])
                        for kt4 in range(4):
                            kt = ktc * 4 + kt4
                            for tt in range(4):
                                nc.tensor.matmul(
                                    psv[tt][:],
                                    xt_sb[:, kt,
                                          vq * 512 + tt * 128:
                                          vq * 512 + (tt + 1) * 128],
                                    wv_c[:, kt4, :],
                                    start=(kt == 0), stop=(kt == KT - 1))
                    for tt in range(4):
                        nc.scalar.copy(vhs[:, vq * 4 + tt, :], psv[tt][:])

            # ---------- Section 2: attention + AllGather + WO ----------
            with (
                tc.tile_pool(name="aq", bufs=4) as aq,
                tc.tile_pool(name="apt", bufs=34) as apt,
                tc.tile_pool(name="awk", bufs=2) as awk,
                tc.tile_pool(name="wop", bufs=1) as wop,
                tc.tile_pool(name="woa", bufs=2) as woa,
                tc.tile_pool(name="woo", bufs=2) as woo,
            ):
                wo_sb = wop.tile([128, KT, DSH], BF16, tag="wo")
                for ch in range(4):
                    wdma = nc.sync.dma_start(
                        wo_sb[:, ch * 8:(ch + 1) * 8, :],
                        wo_d[:, ch * 8:(ch + 1) * 8, :],
                    )
                    tile.add_dep_helper(
                        wdma.ins, anchor[0].ins, sync=False,
                        reason="keep wo_sb load out of the section-1 DMA")

                attn_last_pe = {}
                pts_store = {}
                accs = {}

                def emit_scores(qr, head):
                    kt_h = kts[head]
                    q_sb = aq.tile([128, 512], BF16, tag="qsb",
                                   name=f"qsb{qr}_{head}")
                    nc.scalar.dma_start(
                        q_sb[:],
                        qt_spill[head * 128:(head + 1) * 128,
                                 qr * 512:(qr + 1) * 512])
                    nkt = (4 * qr + 4) if causal else QTILES
                    acc = awk.tile([128, 512], BF16, tag="acc",
                                   name=f"acc{qr}_{head}", bufs=2)
                    pts = []
                    for kt in range(nkt):
                        ps_t = ps.tile([128, 512], F32, tag=f"b{kt % 2}",
                                       name=f"st{qr}_{head}_{kt}", bufs=1)
                        nc.tensor.matmul(
                            ps_t[:], kt_h[:, kt * 128:(kt + 1) * 128],
                            q_sb[:])
                        pT = apt.tile([128, 512], BF16, tag="pT",
                                      name=f"pT{qr}_{head}_{kt}")
                        if general:
                            mt = awk.tile([128, 512], F32, tag="mt")
                            nc.sync.dma_start(
                                mt[:],
                                masktf_d[kt * 128:(kt + 1) * 128,
                                         qr * 512:(qr + 1) * 512])
                            msk = awk.tile([128, 512], F32, tag="msk")
                            nc.vector.scalar_tensor_tensor(
                                msk[:], ps_t[:], SCALE, mt[:],
                                op0=mybir.AluOpType.mult,
                                op1=mybir.AluOpType.add)
                            nc.scalar.activation(
                                pT[:], msk[:],
                                mybir.ActivationFunctionType.Exp)
                        elif not causal or kt < 4 * qr:
                            nc.scalar.activation(
                                pT[:], ps_t[:],
                                mybir.ActivationFunctionType.Exp,
                                scale=SCALE)
                        else:
                            for qtl in range(4):
                                qtile = qr * 4 + qtl
                                blk = slice(qtl * 128, (qtl + 1) * 128)
                                if qtile < kt:
                                    nc.vector.tensor_scalar_mul(
                                        pT[:, blk], ps_t[:, blk], 0.0)
                                elif qtile == kt:
                                    msk = awk.tile([128, 128], F32,
                                                   tag="mskd")
                                    nc.vector.scalar_tensor_tensor(
                                        msk[:], ps_t[:, blk], SCALE,
                                        maskt_sb[:],
                                        op0=mybir.AluOpType.mult,
                                        op1=mybir.AluOpType.add)
                                    nc.scalar.activation(
                                        pT[:, blk], msk[:],
                                        mybir.ActivationFunctionType.Exp)
                                else:
                                    nc.scalar.activation(
                                        pT[:, blk], ps_t[:, blk],
                                        mybir.ActivationFunctionType.Exp,
                                        scale=SCALE)
                        if kt == 0:
                            nc.vector.tensor_copy(acc[:], pT[:])
                        else:
                            nc.vector.tensor_add(acc[:], acc[:], pT[:])
                        pts.append(pT)
                    pts_store[(qr, head)] = pts
                    accs[(qr, head)] = acc

                def emit_pv(qr, head):
                    pts = pts_store.pop((qr, head))
                    acc = accs.pop((qr, head))
                    nkt = len(pts)
                    idx = (qr * 4 + head) % 2
                    ps_pv = ps.tile([128, 512], F32, tag=f"b{2 + idx}",
                                    name=f"pv{qr}_{head}", bufs=1)
                    for kt in range(nkt):
                        nc.tensor.matmul(
                            ps_pv[:],
                            vhs[:, kt, head * 128:(head + 1) * 128],
                            pts[kt][:],
                            start=(kt == 0), stop=(kt == nkt - 1))
                    ps_rsb = ps.tile([128, 512], F32, tag=f"b{3 - idx}",
                                     name=f"rsb{qr}_{head}", bufs=1)
                    rsb_mm = nc.tensor.matmul(ps_rsb[:], onesmat_sb[:],
                                              acc[:])
                    attn_last_pe[(qr, head)] = rsb_mm
                    rec_bc = awk.tile([128, 512], F32, tag="recb", bufs=2)
                    nc.vector.reciprocal_approx_fast(rec_bc[:], ps_rsb[:])
                    at_sb = awk.tile([128, 512], BF16, tag="at")
                    nc.vector.tensor_mul(at_sb[:], ps_pv[:], rec_bc[:])
                    nc.gpsimd.dma_start(
                        agi[qr][head * 128:(head + 1) * 128, :], at_sb[:])

                def emit_ag(qr):
                    nc.gpsimd.collective_compute(
                        "AllGather",
                        mybir.AluOpType.bypass,
                        replica_groups=[list(range(N_CORES))],
                        ins=[agi[qr][:].opt()],
                        outs=[ago[qr][:].opt()],
                    )

                def emit_ag3(hf):
                    nc.gpsimd.collective_compute(
                        "AllGather",
                        mybir.AluOpType.bypass,
                        replica_groups=[list(range(N_CORES))],
                        ins=[agi[3][hf * 256:(hf + 1) * 256, :].opt()],
                        outs=[ago3[hf][:].opt()],
                    )

                # wo_sb rows are host-reordered to [all cores' heads 0-1,
                # then all cores' heads 2-3] so qr3's split halves are
                # contiguous; for qr 0..2 the full-AG output is read with a
                # strided pattern matching that order.
                def emit_wo(r, after=None, halves=(0, 1)):
                    after_inst = attn_last_pe.get(after)
                    if halves[0] == 0:
                        emit_wo._ps[r] = [
                            ps.tile([128, 512], F32, tag=f"a{qtl}",
                                    name=f"wops{r}_{qtl}", bufs=1)
                            for qtl in range(4)]
                    ps_os = emit_wo._ps[r]
                    first_mm = [True]
                    for hf in halves:
                        for cc in range(2):
                            atqf = woa.tile([128, 4, 2, 512], BF16,
                                            tag="atqf",
                                            name=f"atqf{r}_{hf}_{cc}")
                            for hl in range(2):
                                if r < 3:
                                    src = (ago[r]
                                           .rearrange("(c h p) q -> p c h q",
                                                      c=8, h=4)
                                           [:, cc * 4:(cc + 1) * 4,
                                            hf * 2 + hl, :])
                                else:
                                    src = (ago3[hf]
                                           .rearrange("(c h p) q -> p c h q",
                                                      c=8, h=2)
                                           [:, cc * 4:(cc + 1) * 4, hl, :])
                                nc.sync.dma_start(atqf[:, :, hl, :], src)
                            for qtl in range(4):
                                for dt in range(8):
                                    gdt = hf * 16 + cc * 8 + dt
                                    mm = nc.tensor.matmul(
                                        ps_os[qtl][:],
                                        atqf[:, dt // 2, dt % 2,
                                             qtl * 128:(qtl + 1) * 128],
                                        wo_sb[:, gdt, :],
                                        start=(gdt == 0),
                                        stop=(gdt == KT - 1))
                                    if first_mm[0] and after_inst is not None:
                                        tile.add_dep_helper(
                                            mm.ins, after_inst.ins,
                                            sync=False,
                                            reason="order wo after attn")
                                        first_mm[0] = False
                    if halves[-1] == 1:
                        for qtl in range(4):
                            qt = r * 4 + qtl
                            o_sb = woo.tile([128, 512], F32, tag="osb",
                                            name=f"osb{qt}")
                            nc.scalar.copy(o_sb[:], ps_os[qtl][:])
                            nc.sync.dma_start(
                                out_d[qt * 128:(qt + 1) * 128, :], o_sb[:])
                emit_wo._ps = {}

                # ---------- emission schedule ----------
                # software-pipelined: scores(h+1) is emitted before pv(h)
                def emit_qr(qr, mid=None, mid_after=None):
                    emit_scores(qr, 0)
                    emit_scores(qr, 1)
                    emit_pv(qr, 0)
                    if mid is not None:
                        emit_wo(mid, after=mid_after)
                    emit_scores(qr, 2)
                    emit_pv(qr, 1)
                    if qr == 3:
                        emit_ag3_mid()
                    emit_scores(qr, 3)
                    emit_pv(qr, 2)
                    emit_pv(qr, 3)

                def emit_ag3_mid():
                    emit_ag3(0)

                emit_qr(0)
                emit_ag(0)
                emit_qr(1)
                emit_ag(1)
                emit_qr(2, mid=0, mid_after=(2, 0))
                emit_ag(2)
                emit_qr(3, mid=1, mid_after=(3, 0))
                emit_wo(2, after=(3, 3))
                emit_ag3(1)
                emit_wo(3, halves=(0,))
                emit_wo(3, halves=(1,))

    nc.compile()
    return nc


def _get_program(mode):
    if mode not in _PROGRAMS:
        _PROGRAMS[mode] = _build_program(mode)
    return _PROGRAMS[mode]


def _prep_inputs(x, wq, wk, wv, wo, freqs_real, freqs_imag, mask):
    """Host-side shard/layout prep. Returns (mode, in_maps)."""
    x = np.asarray(x, dtype=np.float32)
    wq = np.asarray(wq, dtype=np.float32)
    wk = np.asarray(wk, dtype=np.float32)
    wv = np.asarray(wv, dtype=np.float32)
    wo = np.asarray(wo, dtype=np.float32)
    fr = np.asarray(freqs_real, dtype=np.float32)
    fi = np.asarray(freqs_imag, dtype=np.float32)
    m = np.asarray(mask, dtype=np.float32).reshape(S, S)

    causal_ref = np.triu(np.full((S, S), np.float32(-1e9), dtype=np.float32), k=1)
    if np.array_equal(m, causal_ref):
        mode = "causal"
    elif not m.any():
        mode = "nomask"
    else:
        mode = "general"

    xT = np.ascontiguousarray(x.reshape(S, D).T)  # [D, S]
    xT_bf = xT.astype(ml_dtypes.bfloat16)

    # evens-first permutation of each head's 128 dims (for RoPE pair layout)
    idx = np.concatenate([np.arange(0, HD, 2), np.arange(1, HD, 2)])
    cols = np.concatenate([h * HD + idx for h in range(32)])
    wq_p = wq[:, cols]
    wk_p = wk[:, cols]

    # wo rows reordered: [core c heads 0-1 | core c heads 2-3] blocks, to
    # match the split-AllGather halves of qr3 (and the strided read of the
    # full gathers).
    row_order = np.concatenate(
        [np.arange(c * DSH + hf * 256, c * DSH + (hf + 1) * 256)
         for hf in range(2) for c in range(N_CORES)])
    wo_r = wo[row_order]

    fr128 = np.ascontiguousarray(np.concatenate([fr.T, fr.T], axis=0))   # [128, S]
    fis128 = np.ascontiguousarray(np.concatenate([-fi.T, fi.T], axis=0))

    perm = np.zeros((128, 128), dtype=np.float32)
    perm[np.arange(128), (np.arange(128) + 64) % 128] = 1.0

    onesmat = np.ones((128, 128), dtype=np.float32)

    in_maps = []
    for c in range(N_CORES):
        sl = slice(c * DSH, (c + 1) * DSH)

        def _wtile(a):
            # [D, C] -> [128p, KT, C] matching the SBUF tile layout
            return np.ascontiguousarray(
                a.reshape(KT, 128, a.shape[1]).transpose(1, 0, 2)
            ).astype(ml_dtypes.bfloat16)

        def _whead(a):
            # [D, 512] -> [NH_LOC, 128p, KT, HD]
            return np.ascontiguousarray(np.stack([
                _wtile(a[:, h * HD:(h + 1) * HD]) for h in range(NH_LOC)
            ]))

        im = {
            "xT": xT_bf,
            "wq": _whead(wq_p[:, sl]),
            "wk": _whead(wk_p[:, sl]),
            "wv": _wtile(wv[:, sl]),
            "wo": _wtile(wo_r[:, sl]),
            "fr128": fr128.astype(ml_dtypes.bfloat16),
            "fis128": fis128.astype(ml_dtypes.bfloat16),
            "perm": perm.astype(ml_dtypes.bfloat16),
            "onesmat": onesmat.astype(ml_dtypes.bfloat16),
        }
        if mode == "causal":
            # mask tile in [k, q] layout: valid iff k <= q
            maskt = np.where(
                np.arange(128)[:, None] <= np.arange(128)[None, :],
                np.float32(0.0), np.float32(-1e9)).astype(np.float32)
            im["maskt"] = maskt
        if mode == "general":
            im["masktf"] = np.ascontiguousarray(m.T)
        in_maps.append(im)
    return mode, in_maps


def kernel(x, wq, wk, wv, wo, cache_k, cache_v, freqs_real, freqs_imag,
           mask, start_pos, **_unused):
    assert int(start_pos) == 0, "kernel hardcodes start_pos=0"
    mode, in_maps = _prep_inputs(x, wq, wk, wv, wo, freqs_real, freqs_imag, mask)
    nc = _get_program(mode)
    res = run_bass_kernel_spmd(nc, in_maps, core_ids=list(range(N_CORES)))
    out = np.concatenate([res.results[c]["out"] for c in range(N_CORES)], axis=1)
    return out.reshape(1, S, D).astype(np.float32)


# revision 11
# speedup vs baseline: 1.0067x; 1.0067x over previous
"""Trainium2 Bass kernel for nn_Attention (dense transformer attention block).

Full causal attention: QKV projection + RoPE + softmax(QK^T/sqrt(d) + mask)V + WO,
bsz=1, seqlen=2048, dim=4096, 32 heads x head_dim 128, fp32 I/O.

Sharding: tensor-parallel across heads on 8 NeuronCores. Core c owns heads
4c..4c+3 (wq/wk/wv output columns, attention) and wo output columns
512c..512c+512 (after an AllGather of the per-core attn^T shard along the
head axis). Host concatenates the 8 output column shards.

v2 layout: Q, K AND V projections all run against a fully SBUF-resident x^T
(single 16.8 MB load, consumed at PE rate from the first tile via a
Q0/K0-interleaved start). All matmuls are bf16 (the RoPE half-swap and the
softmax denominator run as bf16 PE matmuls). The final AllGather is split in
two head-halves so the last WO tile overlaps the gather.
"""

import ml_dtypes
import numpy as np

import concourse.bacc as bacc
import concourse.mybir as mybir
import concourse.tile as tile
from concourse.bass_utils import run_bass_kernel_spmd

# Problem constants (hardcoded per contract)
N_CORES = 8
S = 2048              # sequence length
D = 4096              # model dim
HD = 128              # head dim
NH_LOC = 4            # heads per core
DSH = 512             # per-core shard width (NH_LOC * HD)
KT = D // 128         # 32 contraction tiles over model dim
QTILES = S // 128     # 16 token tiles
QRANGES = S // 512    # 4 query ranges of 512
SCALE = float(1.0 / np.sqrt(HD))

F32 = mybir.dt.float32
BF16 = mybir.dt.bfloat16

_PROGRAMS = {}


def _build_program(mode):
    """mode: 'causal' (triu -1e9 mask), 'nomask' (zero mask), 'general'
    (arbitrary additive mask streamed from DRAM)."""
    causal = mode == "causal"
    general = mode == "general"

    nc = bacc.Bacc("TRN2", target_bir_lowering=False, debug=False,
                   num_devices=N_CORES)

    # ---- external inputs (per core) ----
    xT_d = nc.dram_tensor("xT", [D, S], BF16, kind="ExternalInput")
    wq_d = nc.dram_tensor("wq", [NH_LOC, 128, KT, HD], BF16, kind="ExternalInput")
    wk_d = nc.dram_tensor("wk", [NH_LOC, 128, KT, HD], BF16, kind="ExternalInput")
    wv_d = nc.dram_tensor("wv", [128, KT, DSH], BF16, kind="ExternalInput")
    wo_d = nc.dram_tensor("wo", [128, KT, DSH], BF16, kind="ExternalInput")
    fr_d = nc.dram_tensor("fr128", [128, S], BF16, kind="ExternalInput")
    fis_d = nc.dram_tensor("fis128", [128, S], BF16, kind="ExternalInput")
    perm_d = nc.dram_tensor("perm", [128, 128], BF16, kind="ExternalInput")
    onesmat_d = nc.dram_tensor("onesmat", [128, 128], BF16, kind="ExternalInput")
    if causal:
        maskt_d = nc.dram_tensor("maskt", [128, 128], F32, kind="ExternalInput")
    if general:
        masktf_d = nc.dram_tensor("masktf", [S, S], F32, kind="ExternalInput")
    out_d = nc.dram_tensor("out", [S, DSH], F32, kind="ExternalOutput")

    with tile.TileContext(nc) as tc:
        with (
            tc.tile_pool(name="consts", bufs=1) as cns,
            tc.tile_pool(name="dram", bufs=1, space="DRAM") as dram,
            tc.tile_pool(name="akv", bufs=1) as akv,
            tc.tile_pool(name="ps", bufs=1, space="PSUM") as ps,
        ):
            qt_spill = dram.tile([DSH, S], BF16)    # Q^T rotated, [d, s]
            agi = [dram.tile([DSH, 512], BF16, name=f"agi{r}") for r in range(4)]
            # full gathers for qr 0..2; qr 3 split as heads 0-2 / head 3 so
            # the last WO chunk overlaps the last gather
            ago = [dram.tile([D, 512], BF16, addr_space="Shared", name=f"ago{r}")
                   for r in range(3)]
            ago3a = dram.tile([3 * D // 4, 512], BF16, addr_space="Shared",
                              name="ago3a")
            ago3b = dram.tile([D // 4, 512], BF16, addr_space="Shared",
                              name="ago3b")

            onesmat_sb = cns.tile([128, 128], BF16, tag="om")
            if causal:
                maskt_sb = cns.tile([128, 128], F32, tag="maskt")

            kts = [akv.tile([128, S], BF16, tag=f"kth{h}", name=f"kth{h}")
                   for h in range(NH_LOC)]
            # all heads' V: [k-token part, token tile, 4 heads * 128 hd]
            vhs = akv.tile([128, QTILES, DSH], BF16, tag="vhs", name="vhs")

            # ---------- Section 1: Q/K/V projections (x^T fully resident) ----
            anchor = [None]
            with (
                tc.tile_pool(name="p1c", bufs=1) as p1c,
                tc.tile_pool(name="xtp", bufs=1) as xtp,
                tc.tile_pool(name="qkw", bufs=2) as qkw,
                tc.tile_pool(name="qkd", bufs=2) as qkd,
                tc.tile_pool(name="vw", bufs=2) as vw,
            ):
                perm_sb = p1c.tile([128, 128], BF16, tag="perm")
                fr_sb = p1c.tile([128, S], BF16, tag="fr")
                fis_sb = p1c.tile([128, S], BF16, tag="fis")

                xt_sb = xtp.tile([128, KT, S], BF16, tag="xt")
                # fine-grained early chunks so the first matmuls start ~3us in
                xt_chunks = [(0, 1), (1, 2), (2, 3), (3, 4), (4, 6), (6, 8),
                             (8, 11), (11, 15), (15, 20), (20, 26), (26, 32)]
                for ch, (k0, k1) in enumerate(xt_chunks):
                    nc.sync.dma_start(
                        xt_sb[:, k0:k1, :],
                        xT_d[k0 * 128:k1 * 128, :]
                        .rearrange("(kt p) s -> p kt s", p=128),
                    )
                    if ch == 0:
                        nc.scalar.dma_start(perm_sb[:], perm_d[:, :])
                        nc.scalar.dma_start(fr_sb[:], fr_d[:, :])
                        nc.scalar.dma_start(fis_sb[:], fis_d[:, :])
                        nc.scalar.dma_start(onesmat_sb[:], onesmat_d[:, :])
                        if causal:
                            nc.scalar.dma_start(maskt_sb[:], maskt_d[:, :])

                def emit_rope(head, psums, is_q, tagset):
                    """Evacuate 4 psum token-blocks of a projected head,
                    apply RoPE, write Q to the DRAM spill / K to kts."""
                    for j in range(4):
                        qt_bf = qkd.tile([128, 512], BF16, tag="qt")
                        nc.scalar.copy(qt_bf[:], psums[j][:])
                        swap_ps = ps.tile([128, 512], F32, tag=f"{tagset}{j}",
                                          name=f"swap{head}_{is_q}_{j}",
                                          bufs=1)
                        sw = nc.tensor.matmul(swap_ps[:], perm_sb[:], qt_bf[:])
                        if head == 3 and not is_q and j == 3:
                            anchor[0] = sw
                        t1 = qkd.tile([128, 512], F32, tag="t1")
                        nc.vector.tensor_mul(
                            t1[:], qt_bf[:], fr_sb[:, j * 512:(j + 1) * 512])
                        t2 = qkd.tile([128, 512], F32, tag="t2")
                        nc.vector.tensor_mul(
                            t2[:], swap_ps[:],
                            fis_sb[:, j * 512:(j + 1) * 512])
                        if is_q:
                            rot = qkd.tile([128, 512], BF16, tag="rot")
                            nc.vector.tensor_add(rot[:], t1[:], t2[:])
                            nc.sync.dma_start(
                                qt_spill[head * 128:(head + 1) * 128,
                                         j * 512:(j + 1) * 512],
                                rot[:],
                            )
                        else:
                            nc.vector.tensor_add(
                                kts[head][:, j * 512:(j + 1) * 512],
                                t1[:], t2[:])

                # Q/K per head, interleaved per-kt so the PE rides the x^T DMA
                for head in range(NH_LOC):
                    psQ = [ps.tile([128, 512], F32, tag=f"a{j}",
                                   name=f"qps{head}_{j}", bufs=1)
                           for j in range(4)]
                    psK = [ps.tile([128, 512], F32, tag=f"b{j}",
                                   name=f"kps{head}_{j}", bufs=1)
                           for j in range(4)]
                    for wc in range(4):
                        wq_c = qkw.tile([128, 8, 128], BF16, tag="wq")
                        nc.scalar.dma_start(
                            wq_c[:], wq_d[head, :, wc * 8:(wc + 1) * 8, :])
                        wk_c = qkw.tile([128, 8, 128], BF16, tag="wk")
                        nc.scalar.dma_start(
                            wk_c[:], wk_d[head, :, wc * 8:(wc + 1) * 8, :])
                        for kt8 in range(8):
                            kt = wc * 8 + kt8
                            for j in range(4):
                                nc.tensor.matmul(
                                    psQ[j][:], wq_c[:, kt8, :],
                                    xt_sb[:, kt, j * 512:(j + 1) * 512],
                                    start=(kt == 0), stop=(kt == KT - 1))
                            for j in range(4):
                                nc.tensor.matmul(
                                    psK[j][:], wk_c[:, kt8, :],
                                    xt_sb[:, kt, j * 512:(j + 1) * 512],
                                    start=(kt == 0), stop=(kt == KT - 1))
                    emit_rope(head, psQ, True, "a")
                    emit_rope(head, psK, False, "b")

                # V projection from resident x^T
                for vq in range(4):
                    psv = [ps.tile([128, 512], F32, tag=f"a{t}",
                                   name=f"vps{vq}_{t}", bufs=1)
                           for t in range(4)]
                    for ktc in range(8):
                        wv_c = vw.tile([128, 4, 512], BF16, tag="wv")
                        nc.scalar.dma_start(
                            wv_c[:], wv_d[:, ktc * 4:(ktc + 1) * 4, :])
                        for kt4 in range(4):
                            kt = ktc * 4 + kt4
                            for tt in range(4):
                                nc.tensor.matmul(
                                    psv[tt][:],
                                    xt_sb[:, kt,
                                          vq * 512 + tt * 128:
                                          vq * 512 + (tt + 1) * 128],
                                    wv_c[:, kt4, :],
                                    start=(kt == 0), stop=(kt == KT - 1))
                    for tt in range(4):
                        nc.scalar.copy(vhs[:, vq * 4 + tt, :], psv[tt][:])

            # ---------- Section 2: attention + AllGather + WO ----------
            with (
                tc.tile_pool(name="aq", bufs=6) as aq,
                tc.tile_pool(name="apt", bufs=72) as apt,
                tc.tile_pool(name="awk", bufs=2) as awk,
                tc.tile_pool(name="wop", bufs=1) as wop,
                tc.tile_pool(name="woa", bufs=2) as woa,
                tc.tile_pool(name="woo", bufs=2) as woo,
            ):
                wo_sb = wop.tile([128, KT, DSH], BF16, tag="wo")
                for ch in range(4):
                    wdma = nc.sync.dma_start(
                        wo_sb[:, ch * 8:(ch + 1) * 8, :],
                        wo_d[:, ch * 8:(ch + 1) * 8, :],
                    )
                    tile.add_dep_helper(
                        wdma.ins, anchor[0].ins, sync=False,
                        reason="keep wo_sb load out of the section-1 DMA")

                attn_last_pe = {}
                pts_store = {}
                accs = {}

                def emit_scores(qr, head):
                    kt_h = kts[head]
                    q_sb = aq.tile([128, 512], BF16, tag="qsb",
                                   name=f"qsb{qr}_{head}")
                    nc.scalar.dma_start(
                        q_sb[:],
                        qt_spill[head * 128:(head + 1) * 128,
                                 qr * 512:(qr + 1) * 512])
                    nkt = (4 * qr + 4) if causal else QTILES
                    acc = awk.tile([128, 512], BF16, tag="acc",
                                   name=f"acc{qr}_{head}", bufs=4)
                    pts = []
                    for kt in range(nkt):
                        # causal diagonal trim: k-tile kt only affects
                        # queries q >= kt*128, i.e. columns qlo: of this
                        # 512-wide q range
                        delta = kt - 4 * qr if causal else -1
                        qlo = max(0, delta) * 128
                        ps_t = ps.tile([128, 512], F32, tag=f"b{kt % 2}",
                                       name=f"st{qr}_{head}_{kt}", bufs=1)
                        nc.tensor.matmul(
                            ps_t[:, qlo:], kt_h[:, kt * 128:(kt + 1) * 128],
                            q_sb[:, qlo:])
                        pT = apt.tile([128, 512], BF16, tag="pT",
                                      name=f"pT{qr}_{head}_{kt}")
                        if general:
                            mt = awk.tile([128, 512], F32, tag="mt")
                            nc.sync.dma_start(
                                mt[:],
                                masktf_d[kt * 128:(kt + 1) * 128,
                                         qr * 512:(qr + 1) * 512])
                            msk = awk.tile([128, 512], F32, tag="msk")
                            nc.vector.scalar_tensor_tensor(
                                msk[:], ps_t[:], SCALE, mt[:],
                                op0=mybir.AluOpType.mult,
                                op1=mybir.AluOpType.add)
                            nc.scalar.activation(
                                pT[:], msk[:],
                                mybir.ActivationFunctionType.Exp)
                        elif delta < 0:
                            nc.scalar.activation(
                                pT[:], ps_t[:],
                                mybir.ActivationFunctionType.Exp,
                                scale=SCALE)
                        else:
                            for qtl in range(delta, 4):
                                blk = slice(qtl * 128, (qtl + 1) * 128)
                                if qtl == delta:
                                    msk = awk.tile([128, 128], F32,
                                                   tag="mskd")
                                    nc.vector.scalar_tensor_tensor(
                                        msk[:], ps_t[:, blk], SCALE,
                                        maskt_sb[:],
                                        op0=mybir.AluOpType.mult,
                                        op1=mybir.AluOpType.add)
                                    nc.scalar.activation(
                                        pT[:, blk], msk[:],
                                        mybir.ActivationFunctionType.Exp)
                                else:
                                    nc.scalar.activation(
                                        pT[:, blk], ps_t[:, blk],
                                        mybir.ActivationFunctionType.Exp,
                                        scale=SCALE)
                        if kt == 0:
                            nc.vector.tensor_copy(acc[:], pT[:])
                        else:
                            nc.vector.tensor_add(acc[:, qlo:], acc[:, qlo:],
                                                 pT[:, qlo:])
                        pts.append((pT, qlo))
                    pts_store[(qr, head)] = pts
                    accs[(qr, head)] = acc

                def emit_pv(qr, head):
                    pts = pts_store.pop((qr, head))
                    acc = accs.pop((qr, head))
                    nkt = len(pts)
                    idx = (qr * 4 + head) % 2
                    ps_pv = ps.tile([128, 512], F32, tag=f"b{2 + idx}",
                                    name=f"pv{qr}_{head}", bufs=1)
                    for kt in range(nkt):
                        pT, qlo = pts[kt]
                        nc.tensor.matmul(
                            ps_pv[:, qlo:],
                            vhs[:, kt, head * 128:(head + 1) * 128],
                            pT[:, qlo:],
                            start=(kt == 0), stop=(kt == nkt - 1))
                    ps_rsb = ps.tile([128, 512], F32, tag=f"b{3 - idx}",
                                     name=f"rsb{qr}_{head}", bufs=1)
                    rsb_mm = nc.tensor.matmul(ps_rsb[:], onesmat_sb[:],
                                              acc[:])
                    attn_last_pe[(qr, head)] = rsb_mm
                    rec_bc = awk.tile([128, 512], F32, tag="recb", bufs=2)
                    nc.vector.reciprocal_approx_fast(rec_bc[:], ps_rsb[:])
                    at_sb = awk.tile([128, 512], BF16, tag="at", bufs=4)
                    nc.vector.tensor_mul(at_sb[:], ps_pv[:], rec_bc[:])
                    nc.gpsimd.dma_start(
                        agi[qr][head * 128:(head + 1) * 128, :], at_sb[:])

                def emit_ag(qr):
                    nc.gpsimd.collective_compute(
                        "AllGather",
                        mybir.AluOpType.bypass,
                        replica_groups=[list(range(N_CORES))],
                        ins=[agi[qr][:].opt()],
                        outs=[ago[qr][:].opt()],
                    )

                def emit_ag3(hf):
                    if hf == 0:
                        ins, outs = agi[3][0:384, :], ago3a
                    else:
                        ins, outs = agi[3][384:512, :], ago3b
                    nc.gpsimd.collective_compute(
                        "AllGather",
                        mybir.AluOpType.bypass,
                        replica_groups=[list(range(N_CORES))],
                        ins=[ins.opt()],
                        outs=[outs[:].opt()],
                    )

                # wo_sb rows are host-reordered h-major ([head][core][128])
                # so each atqf chunk is one strided DMA and qr3's split
                # gathers are contiguous.
                def emit_wo(r, after=None, hs=(0, 1, 2, 3)):
                    after_inst = attn_last_pe.get(after)
                    if hs[0] == 0:
                        emit_wo._ps[r] = [
                            ps.tile([128, 512], F32, tag=f"a{qtl}",
                                    name=f"wops{r}_{qtl}", bufs=1)
                            for qtl in range(4)]
                    ps_os = emit_wo._ps[r]
                    first_mm = [True]
                    for h in hs:
                        atqf = woa.tile([128, 8, 512], BF16, tag="atqf",
                                        name=f"atqf{r}_{h}")
                        if r < 3:
                            src = (ago[r]
                                   .rearrange("(c h p) q -> p c h q",
                                              c=8, h=4)[:, :, h, :])
                        elif h < 3:
                            src = (ago3a
                                   .rearrange("(c h p) q -> p c h q",
                                              c=8, h=3)[:, :, h, :])
                        else:
                            src = ago3b.rearrange("(c p) q -> p c q", c=8)
                        nc.sync.dma_start(atqf[:], src)
                        for qtl in range(4):
                            for c in range(8):
                                gdt = h * 8 + c
                                mm = nc.tensor.matmul(
                                    ps_os[qtl][:],
                                    atqf[:, c, qtl * 128:(qtl + 1) * 128],
                                    wo_sb[:, gdt, :],
                                    start=(gdt == 0),
                                    stop=(gdt == KT - 1))
                                if first_mm[0] and after_inst is not None:
                                    tile.add_dep_helper(
                                        mm.ins, after_inst.ins,
                                        sync=False,
                                        reason="order wo after attn")
                                    first_mm[0] = False
                    if hs[-1] == 3:
                        for qtl in range(4):
                            qt = r * 4 + qtl
                            o_sb = woo.tile([128, 512], F32, tag="osb",
                                            name=f"osb{qt}")
                            nc.vector.tensor_copy(o_sb[:], ps_os[qtl][:])
                            nc.sync.dma_start(
                                out_d[qt * 128:(qt + 1) * 128, :], o_sb[:])
                emit_wo._ps = {}

                # ---------- emission schedule ----------
                # scores-first per qr so all attention PE work front-loads;
                # WO tiles act as PE filler while the AllGathers run.
                for qr in range(QRANGES):
                    for h in range(NH_LOC):
                        emit_scores(qr, h)
                    for h in range(NH_LOC):
                        emit_pv(qr, h)
                        if qr == 3 and h == 2:
                            emit_ag3(0)
                        if qr == 3 and h == 3:
                            emit_ag3(1)
                    if qr < 3:
                        emit_ag(qr)
                emit_wo(0, after=(3, 3))
                emit_wo(1)
                emit_wo(2)
                emit_wo(3, hs=(0, 1, 2))
                emit_wo(3, hs=(3,))

    nc.compile()
    return nc


def _get_program(mode):
    if mode not in _PROGRAMS:
        _PROGRAMS[mode] = _build_program(mode)
    return _PROGRAMS[mode]


def _prep_inputs(x, wq, wk, wv, wo, freqs_real, freqs_imag, mask):
    """Host-side shard/layout prep. Returns (mode, in_maps)."""
    x = np.asarray(x, dtype=np.float32)
    wq = np.asarray(wq, dtype=np.float32)
    wk = np.asarray(wk, dtype=np.float32)
    wv = np.asarray(wv, dtype=np.float32)
    wo = np.asarray(wo, dtype=np.float32)
    fr = np.asarray(freqs_real, dtype=np.float32)
    fi = np.asarray(freqs_imag, dtype=np.float32)
    m = np.asarray(mask, dtype=np.float32).reshape(S, S)

    causal_ref = np.triu(np.full((S, S), np.float32(-1e9), dtype=np.float32), k=1)
    if np.array_equal(m, causal_ref):
        mode = "causal"
    elif not m.any():
        mode = "nomask"
    else:
        mode = "general"

    xT = np.ascontiguousarray(x.reshape(S, D).T)  # [D, S]
    xT_bf = xT.astype(ml_dtypes.bfloat16)

    # evens-first permutation of each head's 128 dims (for RoPE pair layout)
    idx = np.concatenate([np.arange(0, HD, 2), np.arange(1, HD, 2)])
    cols = np.concatenate([h * HD + idx for h in range(32)])
    wq_p = wq[:, cols]
    wk_p = wk[:, cols]

    # wo rows reordered h-major: [head h][core c][128 dims], matching the
    # AllGather output layout (and qr3's heads-0-2 / head-3 split).
    row_order = np.concatenate(
        [np.arange(c * DSH + h * 128, c * DSH + (h + 1) * 128)
         for h in range(NH_LOC) for c in range(N_CORES)])
    wo_r = wo[row_order]

    fr128 = np.ascontiguousarray(np.concatenate([fr.T, fr.T], axis=0))   # [128, S]
    fis128 = np.ascontiguousarray(np.concatenate([-fi.T, fi.T], axis=0))

    perm = np.zeros((128, 128), dtype=np.float32)
    perm[np.arange(128), (np.arange(128) + 64) % 128] = 1.0

    onesmat = np.ones((128, 128), dtype=np.float32)

    in_maps = []
    for c in range(N_CORES):
        sl = slice(c * DSH, (c + 1) * DSH)

        def _wtile(a):
            # [D, C] -> [128p, KT, C] matching the SBUF tile layout
            return np.ascontiguousarray(
                a.reshape(KT, 128, a.shape[1]).transpose(1, 0, 2)
            ).astype(ml_dtypes.bfloat16)

        def _whead(a):
            # [D, 512] -> [NH_LOC, 128p, KT, HD]
            return np.ascontiguousarray(np.stack([
                _wtile(a[:, h * HD:(h + 1) * HD]) for h in range(NH_LOC)
            ]))

        im = {
            "xT": xT_bf,
            "wq": _whead(wq_p[:, sl]),
            "wk": _whead(wk_p[:, sl]),
            "wv": _wtile(wv[:, sl]),
            "wo": _wtile(wo_r[:, sl]),
            "fr128": fr128.astype(ml_dtypes.bfloat16),
            "fis128": fis128.astype(ml_dtypes.bfloat16),
            "perm": perm.astype(ml_dtypes.bfloat16),
            "onesmat": onesmat.astype(ml_dtypes.bfloat16),
        }
        if mode == "causal":
            # mask tile in [k, q] layout: valid iff k <= q
            maskt = np.where(
                np.arange(128)[:, None] <= np.arange(128)[None, :],
                np.float32(0.0), np.float32(-1e9)).astype(np.float32)
            im["maskt"] = maskt
        if mode == "general":
            im["masktf"] = np.ascontiguousarray(m.T)
        in_maps.append(im)
    return mode, in_maps


def kernel(x, wq, wk, wv, wo, cache_k, cache_v, freqs_real, freqs_imag,
           mask, start_pos, **_unused):
    assert int(start_pos) == 0, "kernel hardcodes start_pos=0"
    mode, in_maps = _prep_inputs(x, wq, wk, wv, wo, freqs_real, freqs_imag, mask)
    nc = _get_program(mode)
    res = run_bass_kernel_spmd(nc, in_maps, core_ids=list(range(N_CORES)))
    out = np.concatenate([res.results[c]["out"] for c in range(N_CORES)], axis=1)
    return out.reshape(1, S, D).astype(np.float32)


# revision 18
# speedup vs baseline: 1.0482x; 1.0412x over previous
"""Trainium2 Bass kernel for nn_Attention (dense transformer attention block).

Full causal attention: QKV projection + RoPE + softmax(QK^T/sqrt(d) + mask)V + WO,
bsz=1, seqlen=2048, dim=4096, 32 heads x head_dim 128, fp32 I/O.

Sharding: tensor-parallel across heads on 8 NeuronCores. Core c owns heads
4c..4c+3 (wq/wk/wv output columns, attention) and wo output columns
512c..512c+512 (after an AllGather of the per-core attn^T shard along the
head axis). Host concatenates the 8 output column shards.

v2 layout: Q, K AND V projections all run against a fully SBUF-resident x^T
(single 16.8 MB load, consumed at PE rate from the first tile via a
Q0/K0-interleaved start). All matmuls are bf16 (the RoPE half-swap and the
softmax denominator run as bf16 PE matmuls). The final AllGather is split in
two head-halves so the last WO tile overlaps the gather.
"""

import ml_dtypes
import numpy as np

import concourse.bacc as bacc
import concourse.mybir as mybir
import concourse.tile as tile
from concourse.bass_utils import run_bass_kernel_spmd

# Problem constants (hardcoded per contract)
N_CORES = 8
S = 2048              # sequence length
D = 4096              # model dim
HD = 128              # head dim
NH_LOC = 4            # heads per core
DSH = 512             # per-core shard width (NH_LOC * HD)
KT = D // 128         # 32 contraction tiles over model dim
QTILES = S // 128     # 16 token tiles
QRANGES = S // 512    # 4 query ranges of 512
SCALE = float(1.0 / np.sqrt(HD))

F32 = mybir.dt.float32
BF16 = mybir.dt.bfloat16

_PROGRAMS = {}


def _build_program(mode):
    """mode: 'causal' (triu -1e9 mask), 'nomask' (zero mask), 'general'
    (arbitrary additive mask streamed from DRAM)."""
    causal = mode == "causal"
    general = mode == "general"

    nc = bacc.Bacc("TRN2", target_bir_lowering=False, debug=False,
                   num_devices=N_CORES)

    # ---- external inputs (per core) ----
    xT_d = nc.dram_tensor("xT", [D, S], BF16, kind="ExternalInput")
    wq_d = nc.dram_tensor("wq", [NH_LOC, 128, KT, HD], BF16, kind="ExternalInput")
    wk_d = nc.dram_tensor("wk", [NH_LOC, 128, KT, HD], BF16, kind="ExternalInput")
    wv_d = nc.dram_tensor("wv", [128, KT, DSH], BF16, kind="ExternalInput")
    wo_d = nc.dram_tensor("wo", [128, KT, DSH], BF16, kind="ExternalInput")
    fr_d = nc.dram_tensor("fr128", [128, S], BF16, kind="ExternalInput")
    fis_d = nc.dram_tensor("fis128", [128, S], BF16, kind="ExternalInput")
    perm_d = nc.dram_tensor("perm", [128, 128], BF16, kind="ExternalInput")
    onesmat_d = nc.dram_tensor("onesmat", [128, 128], BF16, kind="ExternalInput")
    if causal:
        # additive causal mask tile in [k, q] layout, bf16, applied via an
        # accumulating identity-matmul into the scores psum
        maskt_d = nc.dram_tensor("maskt", [128, 128], BF16, kind="ExternalInput")
        ident_d = nc.dram_tensor("ident", [128, 128], BF16, kind="ExternalInput")
    if general:
        masktf_d = nc.dram_tensor("masktf", [S, S], F32, kind="ExternalInput")
    out_d = nc.dram_tensor("out", [S, DSH], F32, kind="ExternalOutput")

    with tile.TileContext(nc) as tc:
        with (
            tc.tile_pool(name="consts", bufs=1) as cns,
            tc.tile_pool(name="dram", bufs=1, space="DRAM") as dram,
            tc.tile_pool(name="akv", bufs=1) as akv,
            tc.tile_pool(name="ps", bufs=1, space="PSUM") as ps,
        ):
            qt_spill = dram.tile([DSH, S], BF16)    # Q^T rotated, [d, s]
            agi = [dram.tile([DSH, 512], BF16, name=f"agi{r}") for r in range(4)]
            # full gathers for qr 0..2; qr 3 split as heads 0-2 / head 3 so
            # the last WO chunk overlaps the last gather
            ago = [dram.tile([D, 512], BF16, addr_space="Shared", name=f"ago{r}")
                   for r in range(3)]
            ago3a = dram.tile([3 * D // 4, 512], BF16, addr_space="Shared",
                              name="ago3a")
            ago3b = dram.tile([D // 4, 512], BF16, addr_space="Shared",
                              name="ago3b")

            onesmat_sb = cns.tile([128, 128], BF16, tag="om")
            if causal:
                maskt_sb = cns.tile([128, 128], BF16, tag="maskt")
                ident_sb = cns.tile([128, 128], BF16, tag="ident")

            kts = [akv.tile([128, S], BF16, tag=f"kth{h}", name=f"kth{h}")
                   for h in range(NH_LOC)]
            # all heads' V: [k-token part, token tile, 4 heads * 128 hd]
            vhs = akv.tile([128, QTILES, DSH], BF16, tag="vhs", name="vhs")

            # ---------- Section 1: Q/K/V projections (x^T fully resident) ----
            anchor = [None]
            with (
                tc.tile_pool(name="p1c", bufs=1) as p1c,
                tc.tile_pool(name="xtp", bufs=1) as xtp,
                tc.tile_pool(name="qkw", bufs=2) as qkw,
                tc.tile_pool(name="qkd", bufs=2) as qkd,
                tc.tile_pool(name="vw", bufs=2) as vw,
            ):
                perm_sb = p1c.tile([128, 128], BF16, tag="perm")
                fr_sb = p1c.tile([128, S], BF16, tag="fr")
                fis_sb = p1c.tile([128, S], BF16, tag="fis")

                xt_sb = xtp.tile([128, KT, S], BF16, tag="xt")
                # fine-grained early chunks so the first matmuls start ~3us in
                xt_chunks = [(0, 1), (1, 2), (2, 3), (3, 4), (4, 6), (6, 8),
                             (8, 11), (11, 15), (15, 20), (20, 26), (26, 32)]
                for ch, (k0, k1) in enumerate(xt_chunks):
                    nc.sync.dma_start(
                        xt_sb[:, k0:k1, :],
                        xT_d[k0 * 128:k1 * 128, :]
                        .rearrange("(kt p) s -> p kt s", p=128),
                    )
                    if ch == 0:
                        nc.gpsimd.dma_start(perm_sb[:], perm_d[:, :])
                        nc.gpsimd.dma_start(fr_sb[:], fr_d[:, :])
                        nc.gpsimd.dma_start(fis_sb[:], fis_d[:, :])
                        nc.gpsimd.dma_start(onesmat_sb[:], onesmat_d[:, :])
                        if causal:
                            nc.gpsimd.dma_start(maskt_sb[:], maskt_d[:, :])
                            nc.gpsimd.dma_start(ident_sb[:], ident_d[:, :])

                def emit_rope(head, psums, is_q, tagset):
                    """Evacuate 4 psum token-blocks of a projected head,
                    apply RoPE, write Q to the DRAM spill / K to kts."""
                    for j in range(4):
                        qt_bf = qkd.tile([128, 512], BF16, tag="qt")
                        nc.scalar.copy(qt_bf[:], psums[j][:])
                        swap_ps = ps.tile([128, 512], F32, tag=f"{tagset}{j}",
                                          name=f"swap{head}_{is_q}_{j}",
                                          bufs=1)
                        sw = nc.tensor.matmul(swap_ps[:], perm_sb[:], qt_bf[:])
                        if head == 3 and not is_q and j == 3:
                            anchor[0] = sw
                        t1 = qkd.tile([128, 512], F32, tag="t1")
                        nc.vector.tensor_mul(
                            t1[:], qt_bf[:], fr_sb[:, j * 512:(j + 1) * 512])
                        t2 = qkd.tile([128, 512], F32, tag="t2")
                        nc.vector.tensor_mul(
                            t2[:], swap_ps[:],
                            fis_sb[:, j * 512:(j + 1) * 512])
                        if is_q:
                            rot = qkd.tile([128, 512], BF16, tag="rot")
                            nc.vector.tensor_add(rot[:], t1[:], t2[:])
                            nc.sync.dma_start(
                                qt_spill[head * 128:(head + 1) * 128,
                                         j * 512:(j + 1) * 512],
                                rot[:],
                            )
                        else:
                            nc.vector.tensor_add(
                                kts[head][:, j * 512:(j + 1) * 512],
                                t1[:], t2[:])

                # Q/K per head, interleaved per-kt so the PE rides the x^T DMA
                for head in range(NH_LOC):
                    psQ = [ps.tile([128, 512], F32, tag=f"a{j}",
                                   name=f"qps{head}_{j}", bufs=1)
                           for j in range(4)]
                    psK = [ps.tile([128, 512], F32, tag=f"b{j}",
                                   name=f"kps{head}_{j}", bufs=1)
                           for j in range(4)]
                    for wc in range(4):
                        wq_c = qkw.tile([128, 8, 128], BF16, tag="wq")
                        nc.scalar.dma_start(
                            wq_c[:], wq_d[head, :, wc * 8:(wc + 1) * 8, :])
                        wk_c = qkw.tile([128, 8, 128], BF16, tag="wk")
                        nc.scalar.dma_start(
                            wk_c[:], wk_d[head, :, wc * 8:(wc + 1) * 8, :])
                        for kt8 in range(8):
                            kt = wc * 8 + kt8
                            for j in range(4):
                                nc.tensor.matmul(
                                    psQ[j][:], wq_c[:, kt8, :],
                                    xt_sb[:, kt, j * 512:(j + 1) * 512],
                                    start=(kt == 0), stop=(kt == KT - 1))
                            for j in range(4):
                                nc.tensor.matmul(
                                    psK[j][:], wk_c[:, kt8, :],
                                    xt_sb[:, kt, j * 512:(j + 1) * 512],
                                    start=(kt == 0), stop=(kt == KT - 1))
                    emit_rope(head, psQ, True, "a")
                    emit_rope(head, psK, False, "b")

                # V projection from resident x^T
                for vq in range(4):
                    psv = [ps.tile([128, 512], F32, tag=f"a{t}",
                                   name=f"vps{vq}_{t}", bufs=1)
                           for t in range(4)]
                    for ktc in range(8):
                        wv_c = vw.tile([128, 4, 512], BF16, tag="wv")
                        nc.scalar.dma_start(
                            wv_c[:], wv_d[:, ktc * 4:(ktc + 1) * 4, :])
                        for kt4 in range(4):
                            kt = ktc * 4 + kt4
                            for tt in range(4):
                                nc.tensor.matmul(
                                    psv[tt][:],
                                    xt_sb[:, kt,
                                          vq * 512 + tt * 128:
                                          vq * 512 + (tt + 1) * 128],
                                    wv_c[:, kt4, :],
                                    start=(kt == 0), stop=(kt == KT - 1))
                    for tt in range(4):
                        nc.scalar.copy(vhs[:, vq * 4 + tt, :], psv[tt][:])

            # ---------- Section 2: attention + AllGather + WO ----------
            with (
                tc.tile_pool(name="aq", bufs=6) as aq,
                tc.tile_pool(name="apt", bufs=72) as apt,
                tc.tile_pool(name="awk", bufs=2) as awk,
                tc.tile_pool(name="wop", bufs=1) as wop,
                tc.tile_pool(name="woa", bufs=2) as woa,
                tc.tile_pool(name="woo", bufs=2) as woo,
            ):
                wo_sb = wop.tile([128, KT, DSH], BF16, tag="wo")
                for ch in range(4):
                    wdma = nc.sync.dma_start(
                        wo_sb[:, ch * 8:(ch + 1) * 8, :],
                        wo_d[:, ch * 8:(ch + 1) * 8, :],
                    )
                    tile.add_dep_helper(
                        wdma.ins, anchor[0].ins, sync=False,
                        reason="keep wo_sb load out of the section-1 DMA")

                attn_last_pe = {}
                pts_store = {}
                accs = {}

                def emit_scores(qr, head):
                    kt_h = kts[head]
                    q_sb = aq.tile([128, 512], BF16, tag="qsb",
                                   name=f"qsb{qr}_{head}")
                    nc.scalar.dma_start(
                        q_sb[:],
                        qt_spill[head * 128:(head + 1) * 128,
                                 qr * 512:(qr + 1) * 512])
                    nkt = (4 * qr + 4) if causal else QTILES
                    acc = awk.tile([128, 512], BF16, tag="acc",
                                   name=f"acc{qr}_{head}", bufs=4)
                    pts = []
                    for kt in range(nkt):
                        # causal diagonal trim: k-tile kt only affects
                        # queries q >= kt*128, i.e. columns qlo: of this
                        # 512-wide q range
                        delta = kt - 4 * qr if causal else -1
                        qlo = max(0, delta) * 128
                        ps_t = ps.tile([128, 512], F32, tag=f"b{kt % 2}",
                                       name=f"st{qr}_{head}_{kt}", bufs=1)
                        nc.tensor.matmul(
                            ps_t[:, qlo:], kt_h[:, kt * 128:(kt + 1) * 128],
                            q_sb[:, qlo:],
                            start=True, stop=(delta < 0))
                        if delta >= 0:
                            # accumulate the triangular -1e9 mask into the
                            # diagonal 128-block: ident.T @ maskt == maskt
                            nc.tensor.matmul(
                                ps_t[:, qlo:qlo + 128], ident_sb[:],
                                maskt_sb[:], start=False, stop=True)
                        pT = apt.tile([128, 512], BF16, tag="pT",
                                      name=f"pT{qr}_{head}_{kt}")
                        if general:
                            mt = awk.tile([128, 512], F32, tag="mt")
                            nc.sync.dma_start(
                                mt[:],
                                masktf_d[kt * 128:(kt + 1) * 128,
                                         qr * 512:(qr + 1) * 512])
                            msk = awk.tile([128, 512], F32, tag="msk")
                            nc.vector.scalar_tensor_tensor(
                                msk[:], ps_t[:], SCALE, mt[:],
                                op0=mybir.AluOpType.mult,
                                op1=mybir.AluOpType.add)
                            nc.scalar.activation(
                                pT[:], msk[:],
                                mybir.ActivationFunctionType.Exp)
                        else:
                            nc.scalar.activation(
                                pT[:, qlo:], ps_t[:, qlo:],
                                mybir.ActivationFunctionType.Exp,
                                scale=SCALE)
                        if kt == 0:
                            nc.vector.tensor_copy(acc[:], pT[:])
                        else:
                            nc.vector.tensor_add(acc[:, qlo:], acc[:, qlo:],
                                                 pT[:, qlo:])
                        pts.append((pT, qlo))
                    pts_store[(qr, head)] = pts
                    accs[(qr, head)] = acc

                def emit_pv(qr, head):
                    pts = pts_store.pop((qr, head))
                    acc = accs.pop((qr, head))
                    nkt = len(pts)
                    idx = (qr * 4 + head) % 2
                    ps_pv = ps.tile([128, 512], F32, tag=f"b{2 + idx}",
                                    name=f"pv{qr}_{head}", bufs=1)
                    for kt in range(nkt):
                        pT, qlo = pts[kt]
                        nc.tensor.matmul(
                            ps_pv[:, qlo:],
                            vhs[:, kt, head * 128:(head + 1) * 128],
                            pT[:, qlo:],
                            start=(kt == 0), stop=(kt == nkt - 1))
                    ps_rsb = ps.tile([128, 512], F32, tag=f"b{3 - idx}",
                                     name=f"rsb{qr}_{head}", bufs=1)
                    rsb_mm = nc.tensor.matmul(ps_rsb[:], onesmat_sb[:],
                                              acc[:])
                    attn_last_pe[(qr, head)] = rsb_mm
                    rec_bc = awk.tile([128, 512], F32, tag="recb", bufs=2)
                    nc.vector.reciprocal_approx_fast(rec_bc[:], ps_rsb[:])
                    at_sb = awk.tile([128, 512], BF16, tag="at", bufs=4)
                    nc.vector.tensor_mul(at_sb[:], ps_pv[:], rec_bc[:])
                    nc.gpsimd.dma_start(
                        agi[qr][head * 128:(head + 1) * 128, :], at_sb[:])

                def emit_ag(qr):
                    nc.gpsimd.collective_compute(
                        "AllGather",
                        mybir.AluOpType.bypass,
                        replica_groups=[list(range(N_CORES))],
                        ins=[agi[qr][:].opt()],
                        outs=[ago[qr][:].opt()],
                    )

                def emit_ag3(hf):
                    if hf == 0:
                        ins, outs = agi[3][0:384, :], ago3a
                    else:
                        ins, outs = agi[3][384:512, :], ago3b
                    nc.gpsimd.collective_compute(
                        "AllGather",
                        mybir.AluOpType.bypass,
                        replica_groups=[list(range(N_CORES))],
                        ins=[ins.opt()],
                        outs=[outs[:].opt()],
                    )

                # wo_sb rows are host-reordered h-major ([head][core][128])
                # so each atqf chunk is one strided DMA and qr3's split
                # gathers are contiguous.
                def emit_wo(r, after=None, hs=(0, 1, 2, 3)):
                    after_inst = attn_last_pe.get(after)
                    if hs[0] == 0:
                        emit_wo._ps[r] = [
                            ps.tile([128, 512], F32, tag=f"a{qtl}",
                                    name=f"wops{r}_{qtl}", bufs=1)
                            for qtl in range(4)]
                    ps_os = emit_wo._ps[r]
                    first_mm = [True]
                    for h in hs:
                        atqf = woa.tile([128, 8, 512], BF16, tag="atqf",
                                        name=f"atqf{r}_{h}")
                        if r < 3:
                            src = (ago[r]
                                   .rearrange("(c h p) q -> p c h q",
                                              c=8, h=4)[:, :, h, :])
                        elif h < 3:
                            src = (ago3a
                                   .rearrange("(c h p) q -> p c h q",
                                              c=8, h=3)[:, :, h, :])
                        else:
                            src = ago3b.rearrange("(c p) q -> p c q", c=8)
                        nc.sync.dma_start(atqf[:], src)
                        for qtl in range(4):
                            for c in range(8):
                                gdt = h * 8 + c
                                mm = nc.tensor.matmul(
                                    ps_os[qtl][:],
                                    atqf[:, c, qtl * 128:(qtl + 1) * 128],
                                    wo_sb[:, gdt, :],
                                    start=(gdt == 0),
                                    stop=(gdt == KT - 1))
                                if first_mm[0] and after_inst is not None:
                                    tile.add_dep_helper(
                                        mm.ins, after_inst.ins,
                                        sync=False,
                                        reason="order wo after attn")
                                    first_mm[0] = False
                    if hs[-1] == 3:
                        for qtl in range(4):
                            qt = r * 4 + qtl
                            o_sb = woo.tile([128, 512], F32, tag="osb",
                                            name=f"osb{qt}")
                            nc.vector.tensor_copy(o_sb[:], ps_os[qtl][:])
                            nc.sync.dma_start(
                                out_d[qt * 128:(qt + 1) * 128, :], o_sb[:])
                emit_wo._ps = {}

                # ---------- emission schedule ----------
                # scores-first per qr so all attention PE work front-loads;
                # WO tiles act as PE filler while the AllGathers run.
                for qr in range(QRANGES):
                    for h in range(NH_LOC):
                        emit_scores(qr, h)
                    for h in range(NH_LOC):
                        emit_pv(qr, h)
                        if qr == 3 and h == 2:
                            emit_ag3(0)
                        if qr == 3 and h == 3:
                            emit_ag3(1)
                    if qr < 3:
                        emit_ag(qr)
                emit_wo(0, after=(3, 3))
                emit_wo(1)
                emit_wo(2)
                emit_wo(3, hs=(0, 1, 2))
                emit_wo(3, hs=(3,))

    nc.compile()
    return nc


def _get_program(mode):
    if mode not in _PROGRAMS:
        _PROGRAMS[mode] = _build_program(mode)
    return _PROGRAMS[mode]


def _prep_inputs(x, wq, wk, wv, wo, freqs_real, freqs_imag, mask):
    """Host-side shard/layout prep. Returns (mode, in_maps)."""
    x = np.asarray(x, dtype=np.float32)
    wq = np.asarray(wq, dtype=np.float32)
    wk = np.asarray(wk, dtype=np.float32)
    wv = np.asarray(wv, dtype=np.float32)
    wo = np.asarray(wo, dtype=np.float32)
    fr = np.asarray(freqs_real, dtype=np.float32)
    fi = np.asarray(freqs_imag, dtype=np.float32)
    m = np.asarray(mask, dtype=np.float32).reshape(S, S)

    causal_ref = np.triu(np.full((S, S), np.float32(-1e9), dtype=np.float32), k=1)
    if np.array_equal(m, causal_ref):
        mode = "causal"
    elif not m.any():
        mode = "nomask"
    else:
        mode = "general"

    xT = np.ascontiguousarray(x.reshape(S, D).T)  # [D, S]
    xT_bf = xT.astype(ml_dtypes.bfloat16)

    # evens-first permutation of each head's 128 dims (for RoPE pair layout)
    idx = np.concatenate([np.arange(0, HD, 2), np.arange(1, HD, 2)])
    cols = np.concatenate([h * HD + idx for h in range(32)])
    wq_p = wq[:, cols]
    wk_p = wk[:, cols]

    # wo rows reordered h-major: [head h][core c][128 dims], matching the
    # AllGather output layout (and qr3's heads-0-2 / head-3 split).
    row_order = np.concatenate(
        [np.arange(c * DSH + h * 128, c * DSH + (h + 1) * 128)
         for h in range(NH_LOC) for c in range(N_CORES)])
    wo_r = wo[row_order]

    fr128 = np.ascontiguousarray(np.concatenate([fr.T, fr.T], axis=0))   # [128, S]
    fis128 = np.ascontiguousarray(np.concatenate([-fi.T, fi.T], axis=0))

    perm = np.zeros((128, 128), dtype=np.float32)
    perm[np.arange(128), (np.arange(128) + 64) % 128] = 1.0

    onesmat = np.ones((128, 128), dtype=np.float32)

    in_maps = []
    for c in range(N_CORES):
        sl = slice(c * DSH, (c + 1) * DSH)

        def _wtile(a):
            # [D, C] -> [128p, KT, C] matching the SBUF tile layout
            return np.ascontiguousarray(
                a.reshape(KT, 128, a.shape[1]).transpose(1, 0, 2)
            ).astype(ml_dtypes.bfloat16)

        def _whead(a):
            # [D, 512] -> [NH_LOC, 128p, KT, HD]
            return np.ascontiguousarray(np.stack([
                _wtile(a[:, h * HD:(h + 1) * HD]) for h in range(NH_LOC)
            ]))

        im = {
            "xT": xT_bf,
            "wq": _whead(wq_p[:, sl]),
            "wk": _whead(wk_p[:, sl]),
            "wv": _wtile(wv[:, sl]),
            "wo": _wtile(wo_r[:, sl]),
            "fr128": fr128.astype(ml_dtypes.bfloat16),
            "fis128": fis128.astype(ml_dtypes.bfloat16),
            "perm": perm.astype(ml_dtypes.bfloat16),
            "onesmat": onesmat.astype(ml_dtypes.bfloat16),
        }
        if mode == "causal":
            # mask tile in [k, q] layout: valid iff k <= q
            maskt = np.where(
                np.arange(128)[:, None] <= np.arange(128)[None, :],
                np.float32(0.0), np.float32(-1e9))
            im["maskt"] = maskt.astype(ml_dtypes.bfloat16)
            im["ident"] = np.eye(128, dtype=np.float32).astype(
                ml_dtypes.bfloat16)
        if mode == "general":
            im["masktf"] = np.ascontiguousarray(m.T)
        in_maps.append(im)
    return mode, in_maps


def kernel(x, wq, wk, wv, wo, cache_k, cache_v, freqs_real, freqs_imag,
           mask, start_pos, **_unused):
    assert int(start_pos) == 0, "kernel hardcodes start_pos=0"
    mode, in_maps = _prep_inputs(x, wq, wk, wv, wo, freqs_real, freqs_imag, mask)
    nc = _get_program(mode)
    res = run_bass_kernel_spmd(nc, in_maps, core_ids=list(range(N_CORES)))
    out = np.concatenate([res.results[c]["out"] for c in range(N_CORES)], axis=1)
    return out.reshape(1, S, D).astype(np.float32)
